# revision 31
# baseline (speedup 1.0000x reference)
"""Trainium2 Bass kernel for nn_B2BConv1d (Hyena-style back-to-back causal
depthwise convs with gating).

Reference computation (B=2, D=4096, L=2048, channels of x are 3*D interleaved
as c = 3*g + p for stream p in {x1, x2, v}):
    features = causal_dw_conv1d(x, w_proj)          # K=3, per-channel weights
    x1, x2, v = de-interleave(features)             # [B, D, L] each
    z = x2 * v
    z = causal_dw_conv1d(z, repeat(w_short, 16))    # K=7, filter shared per 16ch
    out = x1 * z

Sharding: channels (g in [0, 4096)) split across 8 cores, 512 output channels
per core.  No halo needed (convs are along L, fully local per channel).
The host de-interleaves the 3 streams (pure slicing) so each core receives
x1/x2/v shards [2, 512, 2048] plus its per-channel tap weights.

Engine plan (per 128-channel x 2048 unit, bank-tiled at N=512 for PSUM):
  - TensorE: depthwise conv == diagonal-matrix matmul.  For tap k,
    matmul(psum, lhsT=diag(w_k), rhs=x[:, shifted]) accumulates
    w_k[c] * x[c, l-s] into PSUM for free.  f2/fv conv3 and the conv7 run
    here on fp16 operands (fp32 PSUM accumulation).
  - ScalarE (ACT): evacuates fv PSUM->SBUF (fp16) and does the f1 tap-0
    per-partition scale-multiply (fp32).
  - VectorE (DVE): pregate z0 = f2 * fv, f1 taps 1-2 (scalar_tensor_tensor,
    fp32 exact), postgate out = f1 * z.
  - DMA: x1 loaded fp32 (HWDGE); x2/v loaded with fp32->fp16 cast (SWDGE).
"""

import numpy as np
from contextlib import ExitStack

B, D, L = 2, 4096, 2048
NCORES = 8
DG = D // NCORES          # 512 output channels per core
CPT = 128                 # channels per partition tile
NT = DG // CPT            # 4 partition tiles per core
K3, K7 = 3, 7
NB = 4                    # PSUM bank tiles per unit
BW = L // NB              # 512 columns per bank tile

_PROG_CACHE = {}


def build_program(niter=1, variant="full", hwloop=False):
    """Build + compile the (SPMD, per-core) Bass program. Same program runs on
    all 8 cores; only the DRAM input contents differ.

    niter > 1 repeats the whole computation (for wall-clock benchmarking by
    differencing: t(n) - t(1) = (n-1) * t_exec).

    variant: "full" = real kernel; "dmaonly" = same DMA traffic, no compute
    (roofline probe); "nope" = no TensorE convs (f2/fv/z wrong, DMA+DVE+ACT
    only).

    hwloop: wrap the per-pass body in a hardware For_i loop instead of
    unrolling (constant instruction count for any niter -> cheap compiles
    for benchmarking)."""
    import concourse.bacc as bacc
    import concourse.mybir as mybir
    import concourse.tile as tile

    f32 = mybir.dt.float32
    f16 = mybir.dt.float16
    mult = mybir.AluOpType.mult
    add = mybir.AluOpType.add
    Copy = mybir.ActivationFunctionType.Copy

    nc = bacc.Bacc("TRN2", target_bir_lowering=False, debug=False)

    x1d = nc.dram_tensor("x1", [B, DG, L], f32, kind="ExternalInput")
    # x2 and v are packed into one tensor so one SWDGE cast-DMA per unit
    # covers both streams.
    xgd = nc.dram_tensor("xg", [B, 2, DG, L], f32, kind="ExternalInput")
    x2d = xgd[:, 0]
    vd = xgd[:, 1]
    w1d = nc.dram_tensor("w1", [DG, K3], f32, kind="ExternalInput")
    d2d = nc.dram_tensor("d2", [CPT, NT * K3 * CPT], f16, kind="ExternalInput")
    dvd = nc.dram_tensor("dv", [CPT, NT * K3 * CPT], f16, kind="ExternalInput")
    d7d = nc.dram_tensor("d7", [CPT, NT * K7 * CPT], f16, kind="ExternalInput")
    odt = f16 if variant in ("f16out", "b3f16", "b4f16") else f32
    outd = nc.dram_tensor("out", [B, DG, L], odt, kind="ExternalOutput")

    nbuf = {"b3": 3, "b3f16": 3, "b4f16": 4}.get(variant, 2)
    hwcast = variant in ("hwcast", "dmahw", "hwactcast", "hwsplitcast")

    with tile.TileContext(nc) as tc:
        with ExitStack() as ctx:
            wpool = ctx.enter_context(tc.tile_pool(name="wpool", bufs=1))
            xpool = ctx.enter_context(tc.tile_pool(name="xpool", bufs=nbuf))
            mpool = ctx.enter_context(tc.tile_pool(name="mpool", bufs=2))
            opool = ctx.enter_context(tc.tile_pool(name="opool", bufs=nbuf))
            ppool = ctx.enter_context(
                tc.tile_pool(name="ppool", bufs=2, space="PSUM"))

            # f1 per-partition tap weights, one [CPT, K3] block per g-tile.
            w1s = wpool.tile([CPT, NT * K3], f32)
            for gt in range(NT):
                cs = slice(gt * CPT, (gt + 1) * CPT)
                nc.sync.dma_start(w1s[:, gt * K3:(gt + 1) * K3], w1d[cs, :])
            # diag lhsT weight matrices for the PE convs
            d2s = wpool.tile([CPT, NT * K3 * CPT], f16)
            dvs = wpool.tile([CPT, NT * K3 * CPT], f16)
            d7s = wpool.tile([CPT, NT * K7 * CPT], f16)
            nc.sync.dma_start(d2s[:], d2d[:, :])
            nc.sync.dma_start(dvs[:], dvd[:, :])
            nc.sync.dma_start(d7s[:], d7d[:, :])

            def lhsT(dtile, gt, K, k):
                o = (gt * K + k) * CPT
                return dtile[:, o:o + CPT]

            def one_pass():
                for b in range(B):
                    for gt in range(NT):
                        cs = slice(gt * CPT, (gt + 1) * CPT)
                        xt1 = xpool.tile([CPT, 2 + L], f32, tag="xt1")
                        xt2 = xpool.tile([CPT, 2 + L], f16, tag="xt2")
                        xtv = xpool.tile([CPT, 2 + L], f16, tag="xtv")
                        nc.gpsimd.memset(xt1[:, 0:2], 0.0)
                        nc.gpsimd.memset(xt2[:, 0:2], 0.0)
                        nc.gpsimd.memset(xtv[:, 0:2], 0.0)
                        nc.sync.dma_start(xt1[:, 2:2 + L], x1d[b, cs, :])
                        if hwcast:
                            # HWDGE fp32 loads, cast on a compute engine
                            xt2f = xpool.tile([CPT, L], f32, tag="xt2f")
                            xtvf = xpool.tile([CPT, L], f32, tag="xtvf")
                            nc.sync.dma_start(xt2f[:], x2d[b, cs, :])
                            nc.sync.dma_start(xtvf[:], vd[b, cs, :])
                            if variant == "hwcast":
                                nc.gpsimd.tensor_copy(xt2[:, 2:2 + L], xt2f[:])
                                nc.gpsimd.tensor_copy(xtv[:, 2:2 + L], xtvf[:])
                            elif variant == "hwactcast":
                                nc.scalar.activation(
                                    xt2[:, 2:2 + L], xt2f[:], Copy)
                                nc.scalar.activation(
                                    xtv[:, 2:2 + L], xtvf[:], Copy)
                            elif variant == "hwsplitcast":
                                nc.scalar.activation(
                                    xt2[:, 2:2 + L], xt2f[:], Copy)
                                nc.gpsimd.tensor_copy(xtv[:, 2:2 + L], xtvf[:])
                        elif variant == "swchunk":
                            # SWDGE cast DMA, chunked for queue parallelism
                            for q in range(4):
                                c = q * (L // 4)
                                nc.gpsimd.dma_start(
                                    xt2[:, 2 + c:2 + c + L // 4],
                                    x2d[b, cs, c:c + L // 4])
                                nc.gpsimd.dma_start(
                                    xtv[:, 2 + c:2 + c + L // 4],
                                    vd[b, cs, c:c + L // 4])
                        else:
                            # fp32 -> fp16 cast during DMA: SWDGE (gpsimd)
                            # only.  Two dma_starts so they spread across
                            # SWDGE queues and run concurrently.
                            nc.gpsimd.dma_start(xt2[:, 2:2 + L], x2d[b, cs, :])
                            nc.gpsimd.dma_start(xtv[:, 2:2 + L], vd[b, cs, :])

                        if variant in ("dmaonly", "dmahw"):
                            nc.sync.dma_start(outd[b, cs, :], xt1[:, 2:2 + L])
                            continue

                        # f1 path, exact fp32: ACT does tap0, DVE taps 1-2.
                        f1 = mpool.tile([CPT, L], f32, tag="f1")
                        nc.scalar.activation(
                            f1[:], xt1[:, 0:L], Copy,
                            scale=w1s[:, gt * K3:gt * K3 + 1])
                        for k in (1, 2):
                            nc.vector.scalar_tensor_tensor(
                                f1[:], xt1[:, k:k + L],
                                w1s[:, gt * K3 + k:gt * K3 + k + 1], f1[:],
                                mult, add)

                        z0 = mpool.tile([CPT, 6 + L], f16, tag="z0")
                        nc.gpsimd.memset(z0[:, 0:6], 0.0)
                        res = opool.tile([CPT, L], odt, tag="res")

                        for t in range(NB):
                            c0 = t * BW
                            if variant == "nope":
                                nc.vector.tensor_mul(
                                    z0[:, 6 + c0:6 + c0 + BW],
                                    xt2[:, c0:c0 + BW], xtv[:, c0:c0 + BW])
                                fvs = mpool.tile([CPT, BW], f16, tag="fvs")
                                nc.scalar.activation(
                                    fvs[:], z0[:, 6 + c0:6 + c0 + BW], Copy)
                                nc.vector.tensor_mul(
                                    res[:, c0:c0 + BW], fvs[:],
                                    f1[:, c0:c0 + BW])
                                continue
                            pf2 = ppool.tile([CPT, BW], f32, tag="pf2")
                            pfv = ppool.tile([CPT, BW], f32, tag="pfv")
                            # fv first: its PSUM->SBUF evacuation (ACT) can
                            # then overlap the f2 matmuls.
                            for k in range(K3):
                                nc.tensor.matmul(
                                    pfv[:], lhsT(dvs, gt, K3, k),
                                    xtv[:, c0 + k:c0 + k + BW],
                                    start=(k == 0), stop=(k == K3 - 1))
                            for k in range(K3):
                                nc.tensor.matmul(
                                    pf2[:], lhsT(d2s, gt, K3, k),
                                    xt2[:, c0 + k:c0 + k + BW],
                                    start=(k == 0), stop=(k == K3 - 1))
                            fvs = mpool.tile([CPT, BW], f16, tag="fvs")
                            nc.scalar.activation(fvs[:], pfv[:], Copy)
                            nc.vector.tensor_mul(
                                z0[:, 6 + c0:6 + c0 + BW], pf2[:], fvs[:])
                            pz = ppool.tile([CPT, BW], f32, tag="pz")
                            for k in range(K7):
                                nc.tensor.matmul(
                                    pz[:], lhsT(d7s, gt, K7, k),
                                    z0[:, c0 + k:c0 + k + BW],
                                    start=(k == 0), stop=(k == K7 - 1))
                            nc.vector.tensor_mul(
                                res[:, c0:c0 + BW], pz[:], f1[:, c0:c0 + BW])

                        nc.sync.dma_start(outd[b, cs, :], res[:])

            if hwloop and niter > 1:
                with tc.For_i(0, niter, 1):
                    one_pass()
            else:
                for _ in range(niter):
                    one_pass()

    nc.compile()
    return nc


def get_program(niter=1, variant="full", hwloop=False):
    key = ("nc", niter, variant, hwloop)
    if key not in _PROG_CACHE:
        _PROG_CACHE[key] = build_program(niter, variant, hwloop)
    return _PROG_CACHE[key]


def _diag_blocks(w, K):
    """w: [DG, K] fp32 -> [CPT, NT*K*CPT] fp16 with
    out[p, (gt*K+k)*CPT + p] = w[gt*CPT + p, k]."""
    out = np.zeros((CPT, NT * K * CPT), np.float16)
    p = np.arange(CPT)
    for gt in range(NT):
        for k in range(K):
            out[p, (gt * K + k) * CPT + p] = w[gt * CPT:(gt + 1) * CPT,
                                               k].astype(np.float16)
    return out


def make_in_maps(x, w_proj, w_short):
    """Host-side sharding: slice channels across cores and de-interleave the
    3 streams; precompute per-channel tap weight tables."""
    x = np.asarray(x, dtype=np.float32)
    w_proj = np.asarray(w_proj, dtype=np.float32)
    w_short = np.asarray(w_short, dtype=np.float32)
    in_maps = []
    for i in range(NCORES):
        c0 = 3 * DG * i
        xi = x[:, c0:c0 + 3 * DG, :]
        g0 = DG * i
        w2 = w_proj[c0 + 1:c0 + 3 * DG:3, 0, :]
        wv = w_proj[c0 + 2:c0 + 3 * DG:3, 0, :]
        w7 = np.repeat(w_short[g0 // 16:(g0 + DG) // 16, 0, :], 16, axis=0)
        in_maps.append({
            "x1": np.ascontiguousarray(xi[:, 0::3, :]),
            "xg": np.ascontiguousarray(
                np.stack([xi[:, 1::3, :], xi[:, 2::3, :]], axis=1)),
            "w1": np.ascontiguousarray(w_proj[c0 + 0:c0 + 3 * DG:3, 0, :]),
            "d2": _diag_blocks(w2, K3),
            "dv": _diag_blocks(wv, K3),
            "d7": _diag_blocks(w7, K7),
        })
    return in_maps


def kernel(x, w_proj, w_short):
    from concourse.bass_utils import run_bass_kernel_spmd

    nc = get_program(variant="f16out")
    in_maps = make_in_maps(x, w_proj, w_short)
    res = run_bass_kernel_spmd(nc, in_maps, core_ids=list(range(NCORES)))
    out = np.concatenate([res.results[i]["out"] for i in range(NCORES)], axis=1)
    return np.ascontiguousarray(out.astype(np.float32))


# revision 32
# speedup vs baseline: 1.0401x; 1.0401x over previous
"""Trainium2 Bass kernel for nn_B2BConv1d (Hyena-style back-to-back causal
depthwise convs with gating).

Reference computation (B=2, D=4096, L=2048, channels of x are 3*D interleaved
as c = 3*g + p for stream p in {x1, x2, v}):
    features = causal_dw_conv1d(x, w_proj)          # K=3, per-channel weights
    x1, x2, v = de-interleave(features)             # [B, D, L] each
    z = x2 * v
    z = causal_dw_conv1d(z, repeat(w_short, 16))    # K=7, filter shared per 16ch
    out = x1 * z

Sharding: channels (g in [0, 4096)) split across 8 cores, 512 output channels
per core.  No halo needed (convs are along L, fully local per channel).
The host de-interleaves the 3 streams (pure slicing) so each core receives
x1/x2/v shards [2, 512, 2048] plus its per-channel tap weights.

Engine plan (per 128-channel x 2048 unit, bank-tiled at N=512 for PSUM):
  - TensorE: depthwise conv == diagonal-matrix matmul.  For tap k,
    matmul(psum, lhsT=diag(w_k), rhs=x[:, shifted]) accumulates
    w_k[c] * x[c, l-s] into PSUM for free.  f2/fv conv3 and the conv7 run
    here on fp16 operands (fp32 PSUM accumulation).
  - ScalarE (ACT): evacuates fv PSUM->SBUF (fp16) and does the f1 tap-0
    per-partition scale-multiply (fp32).
  - VectorE (DVE): pregate z0 = f2 * fv, f1 taps 1-2 (scalar_tensor_tensor,
    fp32 exact), postgate out = f1 * z.
  - DMA: x1 loaded fp32 (HWDGE); x2/v loaded with fp32->fp16 cast (SWDGE).
"""

import numpy as np
from contextlib import ExitStack

B, D, L = 2, 4096, 2048
NCORES = 8
DG = D // NCORES          # 512 output channels per core
CPT = 128                 # channels per partition tile
NT = DG // CPT            # 4 partition tiles per core
K3, K7 = 3, 7
NB = 4                    # PSUM bank tiles per unit
BW = L // NB              # 512 columns per bank tile

_PROG_CACHE = {}


def build_program(niter=1, variant="full", hwloop=False):
    """Build + compile the (SPMD, per-core) Bass program. Same program runs on
    all 8 cores; only the DRAM input contents differ.

    niter > 1 repeats the whole computation (for wall-clock benchmarking by
    differencing: t(n) - t(1) = (n-1) * t_exec).

    variant: "full" = real kernel; "dmaonly" = same DMA traffic, no compute
    (roofline probe); "nope" = no TensorE convs (f2/fv/z wrong, DMA+DVE+ACT
    only).

    hwloop: wrap the per-pass body in a hardware For_i loop instead of
    unrolling (constant instruction count for any niter -> cheap compiles
    for benchmarking)."""
    import concourse.bacc as bacc
    import concourse.mybir as mybir
    import concourse.tile as tile

    f32 = mybir.dt.float32
    f16 = mybir.dt.float16
    mult = mybir.AluOpType.mult
    add = mybir.AluOpType.add
    Copy = mybir.ActivationFunctionType.Copy

    nc = bacc.Bacc("TRN2", target_bir_lowering=False, debug=False)

    x1d = nc.dram_tensor("x1", [B, DG, L], f32, kind="ExternalInput")
    # x2 and v are packed into one tensor so one SWDGE cast-DMA per unit
    # covers both streams.
    xgd = nc.dram_tensor("xg", [B, 2, DG, L], f32, kind="ExternalInput")
    x2d = xgd[:, 0]
    vd = xgd[:, 1]
    w1d = nc.dram_tensor("w1", [DG, K3], f32, kind="ExternalInput")
    d2d = nc.dram_tensor("d2", [CPT, NT * K3 * CPT], f16, kind="ExternalInput")
    dvd = nc.dram_tensor("dv", [CPT, NT * K3 * CPT], f16, kind="ExternalInput")
    d7d = nc.dram_tensor("d7", [CPT, NT * K7 * CPT], f16, kind="ExternalInput")
    odt = f16 if variant in ("f16out", "b3f16", "b4f16") else f32
    outd = nc.dram_tensor("out", [B, DG, L], odt, kind="ExternalOutput")

    nbuf = {"b3": 3, "b3f16": 3, "b4f16": 4}.get(variant, 2)
    hwcast = variant in ("hwcast", "dmahw", "hwactcast", "hwsplitcast")

    with tile.TileContext(nc) as tc:
        with ExitStack() as ctx:
            wpool = ctx.enter_context(tc.tile_pool(name="wpool", bufs=1))
            xpool = ctx.enter_context(tc.tile_pool(name="xpool", bufs=nbuf))
            mpool = ctx.enter_context(tc.tile_pool(name="mpool", bufs=2))
            opool = ctx.enter_context(tc.tile_pool(name="opool", bufs=nbuf))
            ppool = ctx.enter_context(
                tc.tile_pool(name="ppool", bufs=2, space="PSUM"))

            # f1 per-partition tap weights, one [CPT, K3] block per g-tile.
            w1s = wpool.tile([CPT, NT * K3], f32)
            for gt in range(NT):
                cs = slice(gt * CPT, (gt + 1) * CPT)
                nc.sync.dma_start(w1s[:, gt * K3:(gt + 1) * K3], w1d[cs, :])
            # diag lhsT weight matrices for the PE convs
            d2s = wpool.tile([CPT, NT * K3 * CPT], f16)
            dvs = wpool.tile([CPT, NT * K3 * CPT], f16)
            d7s = wpool.tile([CPT, NT * K7 * CPT], f16)
            nc.sync.dma_start(d2s[:], d2d[:, :])
            nc.sync.dma_start(dvs[:], dvd[:, :])
            nc.sync.dma_start(d7s[:], d7d[:, :])

            def lhsT(dtile, gt, K, k):
                o = (gt * K + k) * CPT
                return dtile[:, o:o + CPT]

            def one_pass():
                for b in range(B):
                    for gt in range(NT):
                        cs = slice(gt * CPT, (gt + 1) * CPT)
                        xt1 = xpool.tile([CPT, 2 + L], f32, tag="xt1")
                        xt2 = xpool.tile([CPT, 2 + L], f16, tag="xt2")
                        xtv = xpool.tile([CPT, 2 + L], f16, tag="xtv")
                        nc.gpsimd.memset(xt1[:, 0:2], 0.0)
                        nc.gpsimd.memset(xt2[:, 0:2], 0.0)
                        nc.gpsimd.memset(xtv[:, 0:2], 0.0)
                        nc.sync.dma_start(xt1[:, 2:2 + L], x1d[b, cs, :])
                        if hwcast:
                            # HWDGE fp32 loads, cast on a compute engine
                            xt2f = xpool.tile([CPT, L], f32, tag="xt2f")
                            xtvf = xpool.tile([CPT, L], f32, tag="xtvf")
                            nc.sync.dma_start(xt2f[:], x2d[b, cs, :])
                            nc.sync.dma_start(xtvf[:], vd[b, cs, :])
                            if variant == "hwcast":
                                nc.gpsimd.tensor_copy(xt2[:, 2:2 + L], xt2f[:])
                                nc.gpsimd.tensor_copy(xtv[:, 2:2 + L], xtvf[:])
                            elif variant == "hwactcast":
                                nc.scalar.activation(
                                    xt2[:, 2:2 + L], xt2f[:], Copy)
                                nc.scalar.activation(
                                    xtv[:, 2:2 + L], xtvf[:], Copy)
                            elif variant == "hwsplitcast":
                                nc.scalar.activation(
                                    xt2[:, 2:2 + L], xt2f[:], Copy)
                                nc.gpsimd.tensor_copy(xtv[:, 2:2 + L], xtvf[:])
                        elif variant == "swchunk":
                            # SWDGE cast DMA, chunked for queue parallelism
                            for q in range(4):
                                c = q * (L // 4)
                                nc.gpsimd.dma_start(
                                    xt2[:, 2 + c:2 + c + L // 4],
                                    x2d[b, cs, c:c + L // 4])
                                nc.gpsimd.dma_start(
                                    xtv[:, 2 + c:2 + c + L // 4],
                                    vd[b, cs, c:c + L // 4])
                        else:
                            # fp32 -> fp16 cast during DMA: SWDGE (gpsimd)
                            # only.  Two dma_starts so they spread across
                            # SWDGE queues and run concurrently.
                            nc.gpsimd.dma_start(xt2[:, 2:2 + L], x2d[b, cs, :])
                            nc.gpsimd.dma_start(xtv[:, 2:2 + L], vd[b, cs, :])

                        if variant in ("dmaonly", "dmahw"):
                            nc.sync.dma_start(outd[b, cs, :], xt1[:, 2:2 + L])
                            continue

                        # f1 path, exact fp32: ACT does tap0, DVE taps 1-2.
                        f1 = mpool.tile([CPT, L], f32, tag="f1")
                        nc.scalar.activation(
                            f1[:], xt1[:, 0:L], Copy,
                            scale=w1s[:, gt * K3:gt * K3 + 1])
                        for k in (1, 2):
                            nc.vector.scalar_tensor_tensor(
                                f1[:], xt1[:, k:k + L],
                                w1s[:, gt * K3 + k:gt * K3 + k + 1], f1[:],
                                mult, add)

                        z0 = mpool.tile([CPT, 6 + L], f16, tag="z0")
                        nc.gpsimd.memset(z0[:, 0:6], 0.0)
                        res = opool.tile([CPT, L], odt, tag="res")

                        for t in range(NB):
                            c0 = t * BW
                            if variant == "nope":
                                nc.vector.tensor_mul(
                                    z0[:, 6 + c0:6 + c0 + BW],
                                    xt2[:, c0:c0 + BW], xtv[:, c0:c0 + BW])
                                fvs = mpool.tile([CPT, BW], f16, tag="fvs")
                                nc.scalar.activation(
                                    fvs[:], z0[:, 6 + c0:6 + c0 + BW], Copy)
                                nc.vector.tensor_mul(
                                    res[:, c0:c0 + BW], fvs[:],
                                    f1[:, c0:c0 + BW])
                                continue
                            pf2 = ppool.tile([CPT, BW], f32, tag="pf2")
                            pfv = ppool.tile([CPT, BW], f32, tag="pfv")
                            # fv first: its PSUM->SBUF evacuation (ACT) can
                            # then overlap the f2 matmuls.
                            for k in range(K3):
                                nc.tensor.matmul(
                                    pfv[:], lhsT(dvs, gt, K3, k),
                                    xtv[:, c0 + k:c0 + k + BW],
                                    start=(k == 0), stop=(k == K3 - 1))
                            for k in range(K3):
                                nc.tensor.matmul(
                                    pf2[:], lhsT(d2s, gt, K3, k),
                                    xt2[:, c0 + k:c0 + k + BW],
                                    start=(k == 0), stop=(k == K3 - 1))
                            fvs = mpool.tile([CPT, BW], f16, tag="fvs")
                            nc.scalar.activation(fvs[:], pfv[:], Copy)
                            nc.vector.tensor_mul(
                                z0[:, 6 + c0:6 + c0 + BW], pf2[:], fvs[:])
                            pz = ppool.tile([CPT, BW], f32, tag="pz")
                            for k in range(K7):
                                nc.tensor.matmul(
                                    pz[:], lhsT(d7s, gt, K7, k),
                                    z0[:, c0 + k:c0 + k + BW],
                                    start=(k == 0), stop=(k == K7 - 1))
                            nc.vector.tensor_mul(
                                res[:, c0:c0 + BW], pz[:], f1[:, c0:c0 + BW])

                        nc.sync.dma_start(outd[b, cs, :], res[:])

            if hwloop and niter > 1:
                with tc.For_i(0, niter, 1):
                    one_pass()
            else:
                for _ in range(niter):
                    one_pass()

    nc.compile()
    return nc


def get_program(niter=1, variant="full", hwloop=False):
    key = ("nc", niter, variant, hwloop)
    if key not in _PROG_CACHE:
        _PROG_CACHE[key] = build_program(niter, variant, hwloop)
    return _PROG_CACHE[key]


def _diag_blocks(w, K):
    """w: [DG, K] fp32 -> [CPT, NT*K*CPT] fp16 with
    out[p, (gt*K+k)*CPT + p] = w[gt*CPT + p, k]."""
    out = np.zeros((CPT, NT * K * CPT), np.float16)
    p = np.arange(CPT)
    for gt in range(NT):
        for k in range(K):
            out[p, (gt * K + k) * CPT + p] = w[gt * CPT:(gt + 1) * CPT,
                                               k].astype(np.float16)
    return out


def make_in_maps(x, w_proj, w_short):
    """Host-side sharding: slice channels across cores and de-interleave the
    3 streams; precompute per-channel tap weight tables."""
    x = np.asarray(x, dtype=np.float32)
    w_proj = np.asarray(w_proj, dtype=np.float32)
    w_short = np.asarray(w_short, dtype=np.float32)
    in_maps = []
    for i in range(NCORES):
        c0 = 3 * DG * i
        xi = x[:, c0:c0 + 3 * DG, :]
        g0 = DG * i
        w2 = w_proj[c0 + 1:c0 + 3 * DG:3, 0, :]
        wv = w_proj[c0 + 2:c0 + 3 * DG:3, 0, :]
        w7 = np.repeat(w_short[g0 // 16:(g0 + DG) // 16, 0, :], 16, axis=0)
        in_maps.append({
            "x1": np.ascontiguousarray(xi[:, 0::3, :]),
            "xg": np.ascontiguousarray(
                np.stack([xi[:, 1::3, :], xi[:, 2::3, :]], axis=1)),
            "w1": np.ascontiguousarray(w_proj[c0 + 0:c0 + 3 * DG:3, 0, :]),
            "d2": _diag_blocks(w2, K3),
            "dv": _diag_blocks(wv, K3),
            "d7": _diag_blocks(w7, K7),
        })
    return in_maps


def kernel(x, w_proj, w_short):
    import os
    from concourse.bass_utils import run_bass_kernel_spmd

    nc = get_program(variant="f16out")
    in_maps = make_in_maps(x, w_proj, w_short)
    try:
        res = run_bass_kernel_spmd(nc, in_maps, core_ids=list(range(NCORES)))
    except ModuleNotFoundError:
        # BASS_TRACE set but this axon client has no NTFF profile hook;
        # rerun with tracing off.
        os.environ["BASS_NEVER_TRACE"] = "1"
        res = run_bass_kernel_spmd(nc, in_maps, core_ids=list(range(NCORES)))
    out = np.concatenate([res.results[i]["out"] for i in range(NCORES)], axis=1)
    return np.ascontiguousarray(out.astype(np.float32))
